# revision 4
# baseline (speedup 1.0000x reference)
"""Multi-head attention (B=4, S=2048, D=1024, H=16, dk=dv=64) on 8 TRN2 cores.

Sharding: core = (batch b, head-group g): data-parallel over batch (4) x
tensor-parallel over heads (2 groups of 8). Each core computes its batch's
Q/K/V projections for its 8 heads, attention, and a partial output
projection over its heads' rows of Wo. The host sums the two partial
outputs per batch.

Per-core kernel (all matmul inputs float32r = TF32-like, fp32 accumulate):
  phase 1: xT streamed in s-tiles of 512; Q/K projected head-pair-packed
           ([128 part] = two heads' 64 dk dims); V projected into
           [s, 8*(64+1)] layout with a ones column per head.
  phase 2: per (head, q-half): scores = Kt.T @ Qt into PSUM [128 s, 1024 q],
           exp on ACT (scale=1/8 folded in) -> SBUF fp32r, AV matmul with
           lhsT=[V|1] accumulating over the 16 s-chunks -> [65, 512] PSUM
           whose row 64 is the softmax denominator. Normalize via DVE
           reciprocal + PE ones-broadcast + DVE multiply.
  phase 3: output projection from the normalized [hv, s] head layout.
"""

import numpy as np

import concourse.bacc as bacc
import concourse.tile as tile
import concourse.mybir as mybir
from concourse.bass_utils import run_bass_kernel_spmd

F32 = mybir.dt.float32
F32R = mybir.dt.float32r
EXP = mybir.ActivationFunctionType.Exp

P = 128
S = 2048
D = 1024
DK = 64
HPC = 8            # heads per core
NSC = S // P       # 16 s-chunks of 128
NST = 4            # s-tiles of 512 for projection
STW = S // NST     # 512
NDC = D // P       # 8 d_model chunks
NPAIR = HPC // 2   # 4 head pairs
QH = 2             # q halves in attention
QW = S // QH       # 1024
SCALE = 1.0 / np.sqrt(DK)


def build_kernel():
    nc = bacc.Bacc("TRN2", target_bir_lowering=False, debug=False)

    xt_d = nc.dram_tensor("xt", [D, S], F32R, kind="ExternalInput")
    wq_d = nc.dram_tensor("wq", [NPAIR, D, P], F32R, kind="ExternalInput")
    wk_d = nc.dram_tensor("wk", [NPAIR, D, P], F32R, kind="ExternalInput")
    wv_d = nc.dram_tensor("wv", [D, HPC * DK], F32R, kind="ExternalInput")
    wo_d = nc.dram_tensor("wo", [HPC * DK, D], F32R, kind="ExternalInput")
    ones_d = nc.dram_tensor("ones", [P, P], F32R, kind="ExternalInput")
    out_d = nc.dram_tensor("out", [S, D], F32, kind="ExternalOutput")

    with tile.TileContext(nc) as tc:
        with tc.tile_pool(name="persist", bufs=1) as persist:
            # Per-pair Q/K in [dk, s] layout: pair p tile holds head 2p at
            # partitions 0:64 and head 2p+1 at 64:128.
            qt = [persist.tile([P, S], F32R, tag=f"qt{p}", name=f"qt{p}")
                  for p in range(NPAIR)]
            kt = [persist.tile([P, S], F32R, tag=f"kt{p}", name=f"kt{p}")
                  for p in range(NPAIR)]
            # V with ones column per head: [128 s, sc, head, 64+1]
            v520 = persist.tile([P, NSC, HPC, DK + 1], F32R, tag="v520")
            ones_lhst = persist.tile([1, DK], F32R, tag="ones_lhst")

            # ---------------- phase 1: projections ----------------
            with tc.tile_pool(name="projw", bufs=1) as projw, \
                 tc.tile_pool(name="projx", bufs=2) as projx, \
                 tc.tile_pool(name="projcp", bufs=4) as projcp, \
                 tc.tile_pool(name="qkps", bufs=4, space="PSUM") as qkps, \
                 tc.tile_pool(name="vps", bufs=3, space="PSUM") as vps:
                wq_sb = projw.tile([P, NDC, NPAIR, P], F32R, tag="wq")
                wk_sb = projw.tile([P, NDC, NPAIR, P], F32R, tag="wk")
                wv_sb = projw.tile([P, NDC, HPC * DK], F32R, tag="wv")
                for pr in range(NPAIR):
                    nc.sync.dma_start(
                        wq_sb[:, :, pr, :],
                        wq_d.ap()[pr].rearrange("(dc p) c -> p dc c", p=P))
                    nc.sync.dma_start(
                        wk_sb[:, :, pr, :],
                        wk_d.ap()[pr].rearrange("(dc p) c -> p dc c", p=P))
                nc.sync.dma_start(
                    wv_sb[:], wv_d.ap().rearrange("(dc p) c -> p dc c", p=P))
                nc.sync.dma_start(ones_lhst[:], ones_d.ap()[0:1, 0:DK])
                for h in range(HPC):
                    nc.sync.dma_start(
                        v520[:, :, h, DK:DK + 1], ones_d.ap()[:, 0:NSC, None])

                xt_ap = xt_d.ap().rearrange("(dc p) s -> p dc s", p=P)
                for st in range(NST):
                    xts = projx.tile([P, NDC, STW], F32R, tag="xts")
                    nc.sync.dma_start(
                        xts[:], xt_ap[:, :, st * STW:(st + 1) * STW])
                    for pr in range(NPAIR):
                        for w_sb, dst in ((wq_sb, qt), (wk_sb, kt)):
                            ps = qkps.tile([P, STW], F32, tag="qkp")
                            for dc in range(NDC):
                                nc.tensor.matmul(
                                    ps[:], w_sb[:, dc, pr, :], xts[:, dc, :],
                                    start=(dc == 0), stop=(dc == NDC - 1))
                            nc.vector.tensor_copy(
                                dst[pr][:, st * STW:(st + 1) * STW], ps[:])
                    for scl in range(STW // P):
                        sc = st * (STW // P) + scl
                        ps = vps.tile([P, HPC * DK], F32, tag="vp")
                        for dc in range(NDC):
                            nc.tensor.matmul(
                                ps[:], xts[:, dc, scl * P:(scl + 1) * P],
                                wv_sb[:, dc, :],
                                start=(dc == 0), stop=(dc == NDC - 1))
                        nc.vector.tensor_copy(
                            v520[:, sc, :, 0:DK],
                            ps.rearrange("p (h v) -> p h v", v=DK))

            # ---------------- phases 2+3 ----------------
            with tc.tile_pool(name="attn", bufs=1) as attn, \
                 tc.tile_pool(name="expp", bufs=3) as expp, \
                 tc.tile_pool(name="smallp", bufs=4) as smallp, \
                 tc.tile_pool(name="bcsb", bufs=2) as bcsb, \
                 tc.tile_pool(name="stage", bufs=4) as stage:
                # normalized heads, [hv, s]: chunk ci = heads (2ci, 2ci+1)
                hn = attn.tile([P, NPAIR, S], F32R, tag="hn")
                wo_sb = attn.tile([P, NPAIR, D], F32R, tag="wo")
                nc.sync.dma_start(
                    wo_sb[:], wo_d.ap().rearrange("(ci p) d -> p ci d", p=P))

                with tc.tile_pool(name="scps", bufs=2, space="PSUM") as scps, \
                     tc.tile_pool(name="avps", bufs=1, space="PSUM") as avps, \
                     tc.tile_pool(name="bcps", bufs=1, space="PSUM") as bcps:
                    for h in range(HPC):
                        pr, j = divmod(h, 2)
                        kt_h = kt[pr][j * DK:(j + 1) * DK, :]
                        qt_h = qt[pr][j * DK:(j + 1) * DK, :]
                        for qh in range(QH):
                            av = [avps.tile([DK + 1, STW], F32, tag=f"av{q}",
                                            name=f"av{q}")
                                  for q in range(QW // STW)]
                            for sc in range(NSC):
                                scp = scps.tile([P, QW], F32, tag="scp")
                                for qq in range(QW // STW):
                                    q0 = qh * QW + qq * STW
                                    nc.tensor.matmul(
                                        scp[:, qq * STW:(qq + 1) * STW],
                                        kt_h[:, sc * P:(sc + 1) * P],
                                        qt_h[:, q0:q0 + STW],
                                        start=True, stop=True)
                                ex = expp.tile([P, QW], F32R, tag="exp")
                                nc.scalar.activation(
                                    ex[:], scp[:], EXP, scale=float(SCALE))
                                for q in range(QW // STW):
                                    nc.tensor.matmul(
                                        av[q][:], v520[:, sc, h, :],
                                        ex[:, q * STW:(q + 1) * STW],
                                        start=(sc == 0), stop=(sc == NSC - 1),
                                        skip_group_check=True)
                            for q in range(QW // STW):
                                den = smallp.tile([1, STW], F32, tag="den")
                                nc.vector.tensor_copy(
                                    den[:], av[q][DK:DK + 1, :])
                                rec = smallp.tile([1, STW], F32R, tag="rec")
                                with nc.allow_low_precision(
                                        reason="softmax recip feeds fp32r mm"):
                                    nc.vector.reciprocal(rec[:], den[:])
                                bcp = bcps.tile([DK, STW], F32, tag="bc")
                                nc.tensor.matmul(
                                    bcp[:], ones_lhst[:], rec[:],
                                    start=True, stop=True)
                                bcs = bcsb.tile([DK, STW], F32, tag="bcs")
                                nc.vector.tensor_copy(bcs[:], bcp[:])
                                q0 = qh * QW + q * STW
                                nc.vector.tensor_mul(
                                    hn[j * DK:(j + 1) * DK, pr, q0:q0 + STW],
                                    av[q][0:DK, :], bcs[:])

                # phase 3: out = hn.T @ wo (contract hv)
                with tc.tile_pool(name="outps", bufs=4, space="PSUM") as outps:
                    for sc in range(NSC):
                        for dmh in range(2):
                            ps = outps.tile([P, D // 2], F32, tag="op")
                            for ci in range(NPAIR):
                                nc.tensor.matmul(
                                    ps[:], hn[:, ci, sc * P:(sc + 1) * P],
                                    wo_sb[:, ci, dmh * 512:(dmh + 1) * 512],
                                    start=(ci == 0), stop=(ci == NPAIR - 1))
                            osb = stage.tile([P, D // 2], F32, tag="ostage")
                            nc.vector.tensor_copy(osb[:], ps[:])
                            nc.sync.dma_start(
                                out_d.ap()[sc * P:(sc + 1) * P,
                                           dmh * 512:(dmh + 1) * 512],
                                osb[:])

    nc.compile()
    return nc


_NC_CACHE = None


def _get_nc():
    global _NC_CACHE
    if _NC_CACHE is None:
        _NC_CACHE = build_kernel()
    return _NC_CACHE


def kernel(x, Wq, Wk, Wv, Wo):
    x = np.asarray(x, dtype=np.float32)
    Wq = np.asarray(Wq, dtype=np.float32)
    Wk = np.asarray(Wk, dtype=np.float32)
    Wv = np.asarray(Wv, dtype=np.float32)
    Wo = np.asarray(Wo, dtype=np.float32)
    B = x.shape[0]
    ones = np.ones((P, P), dtype=np.float32)

    in_maps = []
    for core in range(8):
        b, g = divmod(core, 2)
        hs = g * HPC
        xt = np.ascontiguousarray(x[b].T)
        wq = np.stack([
            np.concatenate([Wq[hs + 2 * p], Wq[hs + 2 * p + 1]], axis=1)
            for p in range(NPAIR)])
        wk = np.stack([
            np.concatenate([Wk[hs + 2 * p], Wk[hs + 2 * p + 1]], axis=1)
            for p in range(NPAIR)])
        wv = np.concatenate([Wv[hs + h] for h in range(HPC)], axis=1)
        wo = np.ascontiguousarray(Wo[hs * DK:(hs + HPC) * DK, :])
        in_maps.append({"xt": xt, "wq": wq, "wk": wk, "wv": wv, "wo": wo,
                        "ones": ones})

    nc = _get_nc()
    res = run_bass_kernel_spmd(nc, in_maps, core_ids=list(range(8))).results

    out = np.empty((B, S, D), dtype=np.float32)
    for b in range(B):
        out[b] = res[2 * b]["out"] + res[2 * b + 1]["out"]
    return out


# revision 15
# speedup vs baseline: 1.2848x; 1.2848x over previous
"""Multi-head attention (B=4, S=2048, D=1024, H=16, dk=dv=64) on 8 TRN2 cores.

Sharding: core = (batch b, head-group g): data-parallel over batch (4) x
tensor-parallel over heads (2 groups of 8). Each core computes its batch's
Q/K/V projections for its 8 heads, attention, and a partial output
projection over its heads' rows of Wo. The host sums the two partial
outputs per batch.

Per-core kernel (matmul inputs float32r = TF32-like, fp32 accumulate):
  V phase:  xT streamed in s-tiles of 512; V for all 8 heads projected into
            a [s, head, 64+1] layout whose per-head ones column later yields
            softmax denominators for free.
  pair loop (4 head pairs): Q/K projected pair-packed (two heads' 64 dk dims
            on partitions 0:64 / 64:128, xT re-streamed), then attention:
            per 512-q-block, both heads' scores land in one [128,1024] PSUM
            tile via matmuls contracting disjoint row-groups (concurrent on
            the PE), one ACT exp (scale=1/8 folded in) covers both, AV
            accumulates per head over 16 s-chunks; row 64 of the AV PSUM is
            the softmax denominator. Normalize = DVE reciprocal + GPSIMD
            partition broadcast + DVE multiply.
  out phase: output projection from the normalized [hv, s] head layout.
"""

import numpy as np

import concourse.bacc as bacc
import concourse.tile as tile
import concourse.mybir as mybir
from concourse.bass_utils import run_bass_kernel_spmd

F32 = mybir.dt.float32
F32R = mybir.dt.float32r
EXP = mybir.ActivationFunctionType.Exp

P = 128
S = 2048
D = 1024
DK = 64
HPC = 8            # heads per core
NSC = S // P       # 16 s-chunks of 128
NST = 4            # s-tiles of 512
STW = S // NST     # 512
NDC = D // P       # 8 d_model chunks
NPAIR = HPC // 2   # 4 head pairs
NQB = S // STW     # 4 q-blocks of 512 in attention
SCALE = 1.0 / np.sqrt(DK)


def build_kernel():
    nc = bacc.Bacc("TRN2", target_bir_lowering=False, debug=False)

    xt_d = nc.dram_tensor("xt", [D, S], F32R, kind="ExternalInput")
    wq_d = nc.dram_tensor("wq", [NPAIR, D, P], F32R, kind="ExternalInput")
    wk_d = nc.dram_tensor("wk", [NPAIR, D, P], F32R, kind="ExternalInput")
    wv_d = nc.dram_tensor("wv", [D, HPC * DK], F32R, kind="ExternalInput")
    wo_d = nc.dram_tensor("wo", [HPC * DK, D], F32R, kind="ExternalInput")
    ones_d = nc.dram_tensor("ones", [P, P], F32R, kind="ExternalInput")
    out_d = nc.dram_tensor("out", [S, D], F32, kind="ExternalOutput")

    xt_ap = xt_d.ap().rearrange("(dc p) s -> p dc s", p=P)

    with tile.TileContext(nc) as tc:
        with tc.tile_pool(name="persist", bufs=1) as persist, \
             tc.tile_pool(name="xtp", bufs=2) as xtp, \
             tc.tile_pool(name="stage", bufs=4) as stage:
            # V with ones column per head: [128 s, sc, head, 64+1]
            v520 = persist.tile([P, NSC, HPC, DK + 1], F32R, tag="v520")
            # normalized heads, [hv, s]: chunk ci = heads (2ci, 2ci+1)
            hn = persist.tile([P, NPAIR, S], F32R, tag="hn")
            wo_sb = persist.tile([P, NPAIR, D], F32R, tag="wo")
            ones_sb = persist.tile([P, HPC], F32R, tag="ones_sb")

            # -------- per-pair: QK projection + attention --------
            # Pair 0's first pass also projects V (shared xT stream), with
            # the pair-0 Q/K matmuls issued first so the first scores/exp
            # start as early as possible.
            with tc.tile_pool(name="wvp", bufs=1) as wvp, \
                 tc.tile_pool(name="qkpool", bufs=2) as qkpool, \
                 tc.tile_pool(name="wqkp", bufs=2) as wqkp, \
                 tc.tile_pool(name="expp", bufs=2) as expp, \
                 tc.tile_pool(name="smallp", bufs=2) as smallp, \
                 tc.tile_pool(name="bcsb", bufs=2) as bcsb, \
                 tc.tile_pool(name="qkps", bufs=2, space="PSUM") as qkps, \
                 tc.tile_pool(name="scps", bufs=2, space="PSUM") as scps, \
                 tc.tile_pool(name="avps", bufs=1, space="PSUM") as avps:
                vps = qkps  # shared double-buffered [128, 512] psum tag
                wv_sb = wvp.tile([P, NDC, HPC * DK], F32R, tag="wv")
                for pr in range(NPAIR):
                    wqp = wqkp.tile([P, NDC, P], F32R, tag="wqp")
                    wkp = wqkp.tile([P, NDC, P], F32R, tag="wkp")
                    nc.sync.dma_start(
                        wqp[:],
                        wq_d.ap()[pr].rearrange("(dc p) c -> p dc c", p=P))
                    nc.sync.dma_start(
                        wkp[:],
                        wk_d.ap()[pr].rearrange("(dc p) c -> p dc c", p=P))
                    if pr == 0:
                        # needed from the first pass, but after pair-0 Q/K
                        nc.sync.dma_start(ones_sb[:], ones_d.ap()[:, 0:HPC])
                        nc.sync.dma_start(
                            wv_sb[:],
                            wv_d.ap().rearrange("(dc p) c -> p dc c", p=P))
                    elif pr == 1:
                        # only needed by the output projection at the end
                        nc.sync.dma_start(
                            wo_sb[:],
                            wo_d.ap().rearrange("(ci p) d -> p ci d", p=P))
                    qtp = qkpool.tile([P, S], F32R, tag="qt")
                    ktp = qkpool.tile([P, S], F32R, tag="kt")

                    def attn_chunk(pr, qb, sc_lo, sc_hi, av,
                                   qtp=qtp, ktp=ktp):
                        q0 = qb * STW
                        for sc in range(sc_lo, sc_hi):
                            scp = scps.tile([P, 2 * STW], F32, tag="scp",
                                            name="scp")
                            for j in range(2):
                                nc.tensor.matmul(
                                    scp[:, j * STW:(j + 1) * STW],
                                    ktp[j * DK:(j + 1) * DK,
                                        sc * P:(sc + 1) * P],
                                    qtp[j * DK:(j + 1) * DK, q0:q0 + STW],
                                    start=True, stop=True)
                            ex = expp.tile([P, 2 * STW], F32R, tag="exp",
                                           name="ex")
                            nc.scalar.activation(
                                ex[:], scp[:], EXP, scale=float(SCALE))
                            for j in range(2):
                                nc.tensor.matmul(
                                    av[j][:], v520[:, sc, 2 * pr + j, :],
                                    ex[:, j * STW:(j + 1) * STW],
                                    start=(sc == 0), stop=(sc == NSC - 1),
                                    skip_group_check=True)

                    def attn_norm(pr, qb, av):
                        q0 = qb * STW
                        for j in range(2):
                            # copy PSUM->SBUF first so the AV bank frees
                            # immediately; normalize off the critical path
                            avs = bcsb.tile([DK + 1, STW], F32, tag="avs",
                                            name="avs")
                            nc.vector.tensor_copy(avs[:], av[j][:])
                            rec = smallp.tile([1, STW], F32R, tag="rec",
                                              name="rec")
                            with nc.allow_low_precision(
                                    reason="softmax recip feeds fp32r mm"):
                                nc.vector.reciprocal(
                                    rec[:], avs[DK:DK + 1, :])
                            bcs = bcsb.tile([DK, STW], F32R, tag="bcs",
                                            name="bcs")
                            nc.gpsimd.partition_broadcast(
                                bcs[:], rec[:], channels=DK)
                            nc.vector.tensor_mul(
                                hn[j * DK:(j + 1) * DK, pr, q0:q0 + STW],
                                avs[0:DK, :], bcs[:])

                    def new_av():
                        return [avps.tile([DK + 1, STW], F32, tag=f"av{j}",
                                          name=f"av{j}")
                                for j in range(2)]

                    av0 = new_av() if pr == 0 else None
                    for st in range(NST):
                        xts = xtp.tile([P, NDC, STW], F32R, tag="xts")
                        nc.sync.dma_start(
                            xts[:], xt_ap[:, :, st * STW:(st + 1) * STW])
                        for w_sb, dst in ((wqp, qtp), (wkp, ktp)):
                            ps = qkps.tile([P, STW], F32, tag="qkp")
                            for dc in range(NDC):
                                nc.tensor.matmul(
                                    ps[:], w_sb[:, dc, :], xts[:, dc, :],
                                    start=(dc == 0), stop=(dc == NDC - 1))
                            nc.vector.tensor_copy(
                                dst[:, st * STW:(st + 1) * STW], ps[:])
                        if pr == 0:
                            # V projection rides pair 0's xT stream
                            for scl in range(STW // P):
                                sc = st * (STW // P) + scl
                                ps = vps.tile([P, HPC * DK], F32, tag="qkp")
                                for dc in range(NDC):
                                    nc.tensor.matmul(
                                        ps[:],
                                        xts[:, dc, scl * P:(scl + 1) * P],
                                        wv_sb[:, dc, :],
                                        start=(dc == 0), stop=(dc == NDC - 1))
                                nc.vector.tensor_copy(
                                    v520[:, sc, :, 0:DK],
                                    ps.rearrange("p (h v) -> p h v", v=DK))
                                nc.vector.tensor_copy(
                                    v520[:, sc, :, DK:DK + 1],
                                    ones_sb[:, :, None])
                            # pair-0 q-block 0 starts as soon as this st's
                            # K/V chunks exist (queries 0:512 are st 0)
                            attn_chunk(0, 0, st * 4, (st + 1) * 4, av0)

                    # attention for this pair, per 512-wide q-block
                    if pr == 0:
                        attn_norm(0, 0, av0)
                    for qb in range(1 if pr == 0 else 0, NQB):
                        av = new_av()
                        attn_chunk(pr, qb, 0, NSC, av)
                        attn_norm(pr, qb, av)

            # ---------------- output projection ----------------
            with tc.tile_pool(name="outps", bufs=4, space="PSUM") as outps:
                for sc in range(NSC):
                    for dmh in range(2):
                        ps = outps.tile([P, D // 2], F32, tag="op")
                        for ci in range(NPAIR):
                            nc.tensor.matmul(
                                ps[:], hn[:, ci, sc * P:(sc + 1) * P],
                                wo_sb[:, ci, dmh * 512:(dmh + 1) * 512],
                                start=(ci == 0), stop=(ci == NPAIR - 1))
                        osb = stage.tile([P, D // 2], F32, tag="ostage")
                        nc.vector.tensor_copy(osb[:], ps[:])
                        nc.sync.dma_start(
                            out_d.ap()[sc * P:(sc + 1) * P,
                                       dmh * 512:(dmh + 1) * 512],
                            osb[:])

    nc.compile()
    return nc


_NC_CACHE = None


def _get_nc():
    global _NC_CACHE
    if _NC_CACHE is None:
        _NC_CACHE = build_kernel()
    return _NC_CACHE


def kernel(x, Wq, Wk, Wv, Wo):
    x = np.asarray(x, dtype=np.float32)
    Wq = np.asarray(Wq, dtype=np.float32)
    Wk = np.asarray(Wk, dtype=np.float32)
    Wv = np.asarray(Wv, dtype=np.float32)
    Wo = np.asarray(Wo, dtype=np.float32)
    B = x.shape[0]
    ones = np.ones((P, P), dtype=np.float32)

    in_maps = []
    for core in range(8):
        b, g = divmod(core, 2)
        hs = g * HPC
        xt = np.ascontiguousarray(x[b].T)
        wq = np.stack([
            np.concatenate([Wq[hs + 2 * p], Wq[hs + 2 * p + 1]], axis=1)
            for p in range(NPAIR)])
        wk = np.stack([
            np.concatenate([Wk[hs + 2 * p], Wk[hs + 2 * p + 1]], axis=1)
            for p in range(NPAIR)])
        wv = np.concatenate([Wv[hs + h] for h in range(HPC)], axis=1)
        wo = np.ascontiguousarray(Wo[hs * DK:(hs + HPC) * DK, :])
        in_maps.append({"xt": xt, "wq": wq, "wk": wk, "wv": wv, "wo": wo,
                        "ones": ones})

    nc = _get_nc()
    res = run_bass_kernel_spmd(nc, in_maps, core_ids=list(range(8))).results

    out = np.empty((B, S, D), dtype=np.float32)
    for b in range(B):
        out[b] = res[2 * b]["out"] + res[2 * b + 1]["out"]
    return out


# revision 18
# speedup vs baseline: 1.3123x; 1.0214x over previous
"""Multi-head attention (B=4, S=2048, D=1024, H=16, dk=dv=64) on 8 TRN2 cores.

Sharding: core = (batch b, head-group g): data-parallel over batch (4) x
tensor-parallel over heads (2 groups of 8). Each core computes its batch's
Q/K/V projections for its 8 heads, attention, and a partial output
projection over its heads' rows of Wo. The host sums the two partial
outputs per batch.

Per-core kernel (matmul inputs float32r = TF32-like, fp32 accumulate):
  V phase:  xT streamed in s-tiles of 512; V for all 8 heads projected into
            a [s, head, 64+1] layout whose per-head ones column later yields
            softmax denominators for free.
  pair loop (4 head pairs): Q/K projected pair-packed (two heads' 64 dk dims
            on partitions 0:64 / 64:128, xT re-streamed), then attention:
            per 512-q-block, both heads' scores land in one [128,1024] PSUM
            tile via matmuls contracting disjoint row-groups (concurrent on
            the PE), one ACT exp (scale=1/8 folded in) covers both, AV
            accumulates per head over 16 s-chunks; row 64 of the AV PSUM is
            the softmax denominator. Normalize = DVE reciprocal + GPSIMD
            partition broadcast + DVE multiply.
  out phase: output projection from the normalized [hv, s] head layout.
"""

import numpy as np

import concourse.bacc as bacc
import concourse.tile as tile
import concourse.mybir as mybir
from concourse.bass_utils import run_bass_kernel_spmd

F32 = mybir.dt.float32
F32R = mybir.dt.float32r
EXP = mybir.ActivationFunctionType.Exp

P = 128
S = 2048
D = 1024
DK = 64
HPC = 8            # heads per core
NSC = S // P       # 16 s-chunks of 128
NST = 4            # s-tiles of 512
STW = S // NST     # 512
NDC = D // P       # 8 d_model chunks
NPAIR = HPC // 2   # 4 head pairs
NQB = S // STW     # 4 q-blocks of 512 in attention
SCALE = 1.0 / np.sqrt(DK)


def build_kernel():
    nc = bacc.Bacc("TRN2", target_bir_lowering=False, debug=False)

    xt_d = nc.dram_tensor("xt", [D, S], F32R, kind="ExternalInput")
    wq_d = nc.dram_tensor("wq", [NPAIR, D, P], F32R, kind="ExternalInput")
    wk_d = nc.dram_tensor("wk", [NPAIR, D, P], F32R, kind="ExternalInput")
    wv_d = nc.dram_tensor("wv", [D, HPC * DK], F32R, kind="ExternalInput")
    wo_d = nc.dram_tensor("wo", [HPC * DK, D], F32R, kind="ExternalInput")
    ones_d = nc.dram_tensor("ones", [P, P], F32R, kind="ExternalInput")
    out_d = nc.dram_tensor("out", [S, D], F32, kind="ExternalOutput")

    xt_ap = xt_d.ap().rearrange("(dc p) s -> p dc s", p=P)

    with tile.TileContext(nc) as tc:
        with tc.tile_pool(name="persist", bufs=1) as persist, \
             tc.tile_pool(name="xtp", bufs=2) as xtp, \
             tc.tile_pool(name="stage", bufs=4) as stage:
            # V with ones column per head: [128 s, sc, head, 64+1]
            v520 = persist.tile([P, NSC, HPC, DK + 1], F32R, tag="v520")
            # normalized heads, [hv, s]: chunk ci = heads (2ci, 2ci+1)
            hn = persist.tile([P, NPAIR, S], F32R, tag="hn")
            wo_sb = persist.tile([P, NPAIR, D], F32R, tag="wo")
            ones_sb = persist.tile([P, HPC], F32R, tag="ones_sb")

            # -------- per-pair: QK projection + attention --------
            # Pair 0's first pass also projects V (shared xT stream), with
            # the pair-0 Q/K matmuls issued first so the first scores/exp
            # start as early as possible.
            with tc.tile_pool(name="wvp", bufs=1) as wvp, \
                 tc.tile_pool(name="qkpool", bufs=2) as qkpool, \
                 tc.tile_pool(name="wqkp", bufs=2) as wqkp, \
                 tc.tile_pool(name="expp", bufs=2) as expp, \
                 tc.tile_pool(name="smallp", bufs=2) as smallp, \
                 tc.tile_pool(name="bcsb", bufs=2) as bcsb, \
                 tc.tile_pool(name="qkps", bufs=2, space="PSUM") as qkps, \
                 tc.tile_pool(name="scps", bufs=2, space="PSUM") as scps, \
                 tc.tile_pool(name="avps", bufs=1, space="PSUM") as avps:
                vps = qkps  # shared double-buffered [128, 512] psum tag
                wv_sb = wvp.tile([P, NDC, HPC * DK], F32R, tag="wv")
                for pr in range(NPAIR):
                    wqp = wqkp.tile([P, NDC, P], F32R, tag="wqp")
                    wkp = wqkp.tile([P, NDC, P], F32R, tag="wkp")
                    nc.sync.dma_start(
                        wqp[:],
                        wq_d.ap()[pr].rearrange("(dc p) c -> p dc c", p=P))
                    nc.sync.dma_start(
                        wkp[:],
                        wk_d.ap()[pr].rearrange("(dc p) c -> p dc c", p=P))
                    if pr == 0:
                        # needed from the first pass, but after pair-0 Q/K
                        nc.sync.dma_start(ones_sb[:], ones_d.ap()[:, 0:HPC])
                        nc.sync.dma_start(
                            wv_sb[:],
                            wv_d.ap().rearrange("(dc p) c -> p dc c", p=P))
                    elif pr == 1:
                        # only needed by the output projection at the end
                        nc.sync.dma_start(
                            wo_sb[:],
                            wo_d.ap().rearrange("(ci p) d -> p ci d", p=P))
                    qtp = qkpool.tile([P, S], F32R, tag="qt")
                    ktp = qkpool.tile([P, S], F32R, tag="kt")

                    def attn_chunk(pr, qb, sc_lo, sc_hi, av,
                                   qtp=qtp, ktp=ktp):
                        q0 = qb * STW
                        for sc in range(sc_lo, sc_hi):
                            scp = scps.tile([P, 2 * STW], F32, tag="scp",
                                            name="scp")
                            for j in range(2):
                                nc.tensor.matmul(
                                    scp[:, j * STW:(j + 1) * STW],
                                    ktp[j * DK:(j + 1) * DK,
                                        sc * P:(sc + 1) * P],
                                    qtp[j * DK:(j + 1) * DK, q0:q0 + STW],
                                    start=True, stop=True)
                            ex = expp.tile([P, 2 * STW], F32R, tag="exp",
                                           name="ex")
                            nc.scalar.activation(
                                ex[:], scp[:], EXP, scale=float(SCALE))
                            for j in range(2):
                                nc.tensor.matmul(
                                    av[j][:], v520[:, sc, 2 * pr + j, :],
                                    ex[:, j * STW:(j + 1) * STW],
                                    start=(sc == 0), stop=(sc == NSC - 1),
                                    skip_group_check=True)

                    def attn_norm(pr, qb, av):
                        q0 = qb * STW
                        for j in range(2):
                            # copy PSUM->SBUF first so the AV bank frees
                            # immediately; normalize off the critical path
                            avs = bcsb.tile([DK + 1, STW], F32, tag="avs",
                                            name="avs")
                            nc.vector.tensor_copy(avs[:], av[j][:])
                            rec = smallp.tile([1, STW], F32R, tag="rec",
                                              name="rec")
                            with nc.allow_low_precision(
                                    reason="softmax recip feeds fp32r mm"):
                                nc.vector.reciprocal(
                                    rec[:], avs[DK:DK + 1, :])
                            bcs = bcsb.tile([DK, STW], F32R, tag="bcs",
                                            name="bcs")
                            nc.gpsimd.partition_broadcast(
                                bcs[:], rec[:], channels=DK)
                            nc.vector.tensor_mul(
                                hn[j * DK:(j + 1) * DK, pr, q0:q0 + STW],
                                avs[0:DK, :], bcs[:])

                    def new_av():
                        return [avps.tile([DK + 1, STW], F32, tag=f"av{j}",
                                          name=f"av{j}")
                                for j in range(2)]

                    av0 = new_av() if pr == 0 else None
                    for st in range(NST):
                        xts = xtp.tile([P, NDC, STW], F32R, tag="xts")
                        for dh in range(0, NDC, 2):
                            nc.sync.dma_start(
                                xts[:, dh:dh + 2, :],
                                xt_ap[:, dh:dh + 2,
                                      st * STW:(st + 1) * STW])
                        for w_sb, dst in ((wqp, qtp), (wkp, ktp)):
                            ps = qkps.tile([P, STW], F32, tag="qkp")
                            for dc in range(NDC):
                                nc.tensor.matmul(
                                    ps[:], w_sb[:, dc, :], xts[:, dc, :],
                                    start=(dc == 0), stop=(dc == NDC - 1))
                            nc.vector.tensor_copy(
                                dst[:, st * STW:(st + 1) * STW], ps[:])
                        if pr == 0:
                            # V projection rides pair 0's xT stream
                            for scl in range(STW // P):
                                sc = st * (STW // P) + scl
                                ps = vps.tile([P, HPC * DK], F32, tag="qkp")
                                for dc in range(NDC):
                                    nc.tensor.matmul(
                                        ps[:],
                                        xts[:, dc, scl * P:(scl + 1) * P],
                                        wv_sb[:, dc, :],
                                        start=(dc == 0), stop=(dc == NDC - 1))
                                nc.vector.tensor_copy(
                                    v520[:, sc, :, 0:DK],
                                    ps.rearrange("p (h v) -> p h v", v=DK))
                                nc.vector.tensor_copy(
                                    v520[:, sc, :, DK:DK + 1],
                                    ones_sb[:, :, None])
                            # pair-0 q-block 0 starts as soon as this st's
                            # K/V chunks exist (queries 0:512 are st 0)
                            attn_chunk(0, 0, st * 4, (st + 1) * 4, av0)

                    # attention for this pair, per 512-wide q-block
                    if pr == 0:
                        attn_norm(0, 0, av0)
                    for qb in range(1 if pr == 0 else 0, NQB):
                        av = new_av()
                        attn_chunk(pr, qb, 0, NSC, av)
                        attn_norm(pr, qb, av)

            # ---------------- output projection ----------------
            with tc.tile_pool(name="outps", bufs=4, space="PSUM") as outps:
                for sc in range(NSC):
                    for dmh in range(2):
                        ps = outps.tile([P, D // 2], F32, tag="op")
                        for ci in range(NPAIR):
                            nc.tensor.matmul(
                                ps[:], hn[:, ci, sc * P:(sc + 1) * P],
                                wo_sb[:, ci, dmh * 512:(dmh + 1) * 512],
                                start=(ci == 0), stop=(ci == NPAIR - 1))
                        osb = stage.tile([P, D // 2], F32, tag="ostage")
                        nc.vector.tensor_copy(osb[:], ps[:])
                        nc.sync.dma_start(
                            out_d.ap()[sc * P:(sc + 1) * P,
                                       dmh * 512:(dmh + 1) * 512],
                            osb[:])

    nc.compile()
    return nc


_NC_CACHE = None


def _get_nc():
    global _NC_CACHE
    if _NC_CACHE is None:
        _NC_CACHE = build_kernel()
    return _NC_CACHE


def kernel(x, Wq, Wk, Wv, Wo):
    x = np.asarray(x, dtype=np.float32)
    Wq = np.asarray(Wq, dtype=np.float32)
    Wk = np.asarray(Wk, dtype=np.float32)
    Wv = np.asarray(Wv, dtype=np.float32)
    Wo = np.asarray(Wo, dtype=np.float32)
    B = x.shape[0]
    ones = np.ones((P, P), dtype=np.float32)

    in_maps = []
    for core in range(8):
        b, g = divmod(core, 2)
        hs = g * HPC
        xt = np.ascontiguousarray(x[b].T)
        wq = np.stack([
            np.concatenate([Wq[hs + 2 * p], Wq[hs + 2 * p + 1]], axis=1)
            for p in range(NPAIR)])
        wk = np.stack([
            np.concatenate([Wk[hs + 2 * p], Wk[hs + 2 * p + 1]], axis=1)
            for p in range(NPAIR)])
        wv = np.concatenate([Wv[hs + h] for h in range(HPC)], axis=1)
        wo = np.ascontiguousarray(Wo[hs * DK:(hs + HPC) * DK, :])
        in_maps.append({"xt": xt, "wq": wq, "wk": wk, "wv": wv, "wo": wo,
                        "ones": ones})

    nc = _get_nc()
    res = run_bass_kernel_spmd(nc, in_maps, core_ids=list(range(8))).results

    out = np.empty((B, S, D), dtype=np.float32)
    for b in range(B):
        out[b] = res[2 * b]["out"] + res[2 * b + 1]["out"]
    return out


# revision 21
# speedup vs baseline: 1.3418x; 1.0225x over previous
"""Multi-head attention (B=4, S=2048, D=1024, H=16, dk=dv=64) on 8 TRN2 cores.

Sharding: core = (batch b, head-group g): data-parallel over batch (4) x
tensor-parallel over heads (2 groups of 8). Each core computes its batch's
Q/K/V projections for its 8 heads, attention, and a partial output
projection over its heads' rows of Wo. The host sums the two partial
outputs per batch.

Per-core kernel (matmul inputs float32r = TF32-like, fp32 accumulate):
  V phase:  xT streamed in s-tiles of 512; V for all 8 heads projected into
            a [s, head, 64+1] layout whose per-head ones column later yields
            softmax denominators for free.
  pair loop (4 head pairs): Q/K projected pair-packed (two heads' 64 dk dims
            on partitions 0:64 / 64:128, xT re-streamed), then attention:
            per 512-q-block, both heads' scores land in one [128,1024] PSUM
            tile via matmuls contracting disjoint row-groups (concurrent on
            the PE), one ACT exp (scale=1/8 folded in) covers both, AV
            accumulates per head over 16 s-chunks; row 64 of the AV PSUM is
            the softmax denominator. Normalize = DVE reciprocal + GPSIMD
            partition broadcast + DVE multiply.
  out phase: output projection from the normalized [hv, s] head layout.
"""

import numpy as np

import concourse.bacc as bacc
import concourse.tile as tile
import concourse.mybir as mybir
from concourse.bass_utils import run_bass_kernel_spmd

F32 = mybir.dt.float32
F32R = mybir.dt.float32r
EXP = mybir.ActivationFunctionType.Exp

P = 128
S = 2048
D = 1024
DK = 64
HPC = 8            # heads per core
NSC = S // P       # 16 s-chunks of 128
NST = 4            # s-tiles of 512
STW = S // NST     # 512
NDC = D // P       # 8 d_model chunks
NPAIR = HPC // 2   # 4 head pairs
NQB = S // STW     # 4 q-blocks of 512 in attention
SCALE = 1.0 / np.sqrt(DK)


def build_kernel():
    nc = bacc.Bacc("TRN2", target_bir_lowering=False, debug=False)

    xt_d = nc.dram_tensor("xt", [D, S], F32R, kind="ExternalInput")
    wq_d = nc.dram_tensor("wq", [NPAIR, D, P], F32R, kind="ExternalInput")
    wk_d = nc.dram_tensor("wk", [NPAIR, D, P], F32R, kind="ExternalInput")
    wv_d = nc.dram_tensor("wv", [D, HPC * DK], F32R, kind="ExternalInput")
    wo_d = nc.dram_tensor("wo", [HPC * DK, D], F32R, kind="ExternalInput")
    ones_d = nc.dram_tensor("ones", [P, P], F32R, kind="ExternalInput")
    out_d = nc.dram_tensor("out", [S, D], F32, kind="ExternalOutput")

    xt_ap = xt_d.ap().rearrange("(dc p) s -> p dc s", p=P)

    with tile.TileContext(nc) as tc:
        with tc.tile_pool(name="persist", bufs=1) as persist, \
             tc.tile_pool(name="xtp", bufs=2) as xtp, \
             tc.tile_pool(name="stage", bufs=4) as stage:
            # V with ones column per head: [128 s, sc, head, 64+1]
            v520 = persist.tile([P, NSC, HPC, DK + 1], F32R, tag="v520")
            # normalized heads, [hv, s]: chunk ci = heads (2ci, 2ci+1)
            hn = persist.tile([P, NPAIR, S], F32R, tag="hn")
            wo_sb = persist.tile([P, NPAIR, D], F32R, tag="wo")
            ones_sb = persist.tile([P, HPC], F32R, tag="ones_sb")

            # -------- per-pair: QK projection + attention --------
            # Pair 0's first pass also projects V (shared xT stream), with
            # the pair-0 Q/K matmuls issued first so the first scores/exp
            # start as early as possible.
            with tc.tile_pool(name="wvp", bufs=1) as wvp, \
                 tc.tile_pool(name="qkpool", bufs=2) as qkpool, \
                 tc.tile_pool(name="wqkp", bufs=2) as wqkp, \
                 tc.tile_pool(name="expp", bufs=2) as expp, \
                 tc.tile_pool(name="smallp", bufs=2) as smallp, \
                 tc.tile_pool(name="bcsb", bufs=2) as bcsb, \
                 tc.tile_pool(name="qkps", bufs=2, space="PSUM") as qkps, \
                 tc.tile_pool(name="scps", bufs=2, space="PSUM") as scps, \
                 tc.tile_pool(name="avps", bufs=1, space="PSUM") as avps:
                vps = qkps  # shared double-buffered [128, 512] psum tag
                wv_sb = wvp.tile([P, NDC, HPC * DK], F32R, tag="wv")
                for pr in range(NPAIR):
                    wqp = wqkp.tile([P, NDC, P], F32R, tag="wqp")
                    wkp = wqkp.tile([P, NDC, P], F32R, tag="wkp")
                    nc.sync.dma_start(
                        wqp[:],
                        wq_d.ap()[pr].rearrange("(dc p) c -> p dc c", p=P))
                    nc.sync.dma_start(
                        wkp[:],
                        wk_d.ap()[pr].rearrange("(dc p) c -> p dc c", p=P))
                    if pr == 0:
                        # needed from the first pass, but after pair-0 Q/K
                        nc.sync.dma_start(ones_sb[:], ones_d.ap()[:, 0:HPC])
                        nc.sync.dma_start(
                            wv_sb[:],
                            wv_d.ap().rearrange("(dc p) c -> p dc c", p=P))
                    elif pr == 1:
                        # only needed by the output projection at the end
                        nc.sync.dma_start(
                            wo_sb[:],
                            wo_d.ap().rearrange("(ci p) d -> p ci d", p=P))
                    qtp = qkpool.tile([P, S], F32R, tag="qt")
                    ktp = qkpool.tile([P, S], F32R, tag="kt")

                    def out_group(sc, dmh):
                        # one output-projection psum group ([128, 512] out
                        # rows sc, cols dmh-half); reuses the qkp PSUM banks
                        ps = qkps.tile([P, D // 2], F32, tag="qkp",
                                       name="ops")
                        for ci in range(NPAIR):
                            nc.tensor.matmul(
                                ps[:], hn[:, ci, sc * P:(sc + 1) * P],
                                wo_sb[:, ci, dmh * 512:(dmh + 1) * 512],
                                start=(ci == 0), stop=(ci == NPAIR - 1))
                        osb = stage.tile([P, D // 2], F32,
                                         tag="ostage", name="osb")
                        nc.vector.tensor_copy(osb[:], ps[:])
                        nc.sync.dma_start(
                            out_d.ap()[sc * P:(sc + 1) * P,
                                       dmh * 512:(dmh + 1) * 512],
                            osb[:])

                    def attn_chunk(pr, qb, sc_lo, sc_hi, av, trail=None,
                                   qtp=qtp, ktp=ktp):
                        q0 = qb * STW
                        for sc in range(sc_lo, sc_hi):
                            scp = scps.tile([P, 2 * STW], F32, tag="scp",
                                            name="scp")
                            for j in range(2):
                                nc.tensor.matmul(
                                    scp[:, j * STW:(j + 1) * STW],
                                    ktp[j * DK:(j + 1) * DK,
                                        sc * P:(sc + 1) * P],
                                    qtp[j * DK:(j + 1) * DK, q0:q0 + STW],
                                    start=True, stop=True)
                            ex = expp.tile([P, 2 * STW], F32R, tag="exp",
                                           name="ex")
                            nc.scalar.activation(
                                ex[:], scp[:], EXP, scale=float(SCALE))
                            for j in range(2):
                                nc.tensor.matmul(
                                    av[j][:], v520[:, sc, 2 * pr + j, :],
                                    ex[:, j * STW:(j + 1) * STW],
                                    start=(sc == 0), stop=(sc == NSC - 1),
                                    skip_group_check=True)
                            if trail and sc % 2 == 1:
                                out_group(*trail.pop(0))

                    def attn_norm(pr, qb, av):
                        q0 = qb * STW
                        for j in range(2):
                            # copy PSUM->SBUF first so the AV bank frees
                            # immediately; normalize off the critical path
                            avs = bcsb.tile([DK + 1, STW], F32, tag="avs",
                                            name="avs")
                            nc.vector.tensor_copy(avs[:], av[j][:])
                            rec = smallp.tile([1, STW], F32R, tag="rec",
                                              name="rec")
                            with nc.allow_low_precision(
                                    reason="softmax recip feeds fp32r mm"):
                                nc.vector.reciprocal(
                                    rec[:], avs[DK:DK + 1, :])
                            bcs = bcsb.tile([DK, STW], F32R, tag="bcs",
                                            name="bcs")
                            nc.gpsimd.partition_broadcast(
                                bcs[:], rec[:], channels=DK)
                            nc.vector.tensor_mul(
                                hn[j * DK:(j + 1) * DK, pr, q0:q0 + STW],
                                avs[0:DK, :], bcs[:])

                    def new_av():
                        return [avps.tile([DK + 1, STW], F32, tag=f"av{j}",
                                          name=f"av{j}")
                                for j in range(2)]

                    av0 = new_av() if pr == 0 else None
                    for st in range(NST):
                        xts = xtp.tile([P, NDC, STW], F32R, tag="xts")
                        for dh in range(0, NDC, 2):
                            nc.sync.dma_start(
                                xts[:, dh:dh + 2, :],
                                xt_ap[:, dh:dh + 2,
                                      st * STW:(st + 1) * STW])
                        for w_sb, dst in ((wqp, qtp), (wkp, ktp)):
                            ps = qkps.tile([P, STW], F32, tag="qkp")
                            for dc in range(NDC):
                                nc.tensor.matmul(
                                    ps[:], w_sb[:, dc, :], xts[:, dc, :],
                                    start=(dc == 0), stop=(dc == NDC - 1))
                            nc.vector.tensor_copy(
                                dst[:, st * STW:(st + 1) * STW], ps[:])
                        if pr == 0:
                            # V projection rides pair 0's xT stream
                            for scl in range(STW // P):
                                sc = st * (STW // P) + scl
                                ps = vps.tile([P, HPC * DK], F32, tag="qkp")
                                for dc in range(NDC):
                                    nc.tensor.matmul(
                                        ps[:],
                                        xts[:, dc, scl * P:(scl + 1) * P],
                                        wv_sb[:, dc, :],
                                        start=(dc == 0), stop=(dc == NDC - 1))
                                nc.vector.tensor_copy(
                                    v520[:, sc, :, 0:DK],
                                    ps.rearrange("p (h v) -> p h v", v=DK))
                                nc.vector.tensor_copy(
                                    v520[:, sc, :, DK:DK + 1],
                                    ones_sb[:, :, None])
                            # pair-0 q-block 0 starts as soon as this st's
                            # K/V chunks exist (queries 0:512 are st 0)
                            attn_chunk(0, 0, st * 4, (st + 1) * 4, av0)

                    # attention for this pair, per 512-wide q-block. For the
                    # last pair, q-block qb-1 is complete once norm(qb-1)
                    # ran, so its output-projection groups interleave into
                    # attention of qb (one group per two s-chunks).
                    if pr == 0:
                        attn_norm(0, 0, av0)
                    for qb in range(1 if pr == 0 else 0, NQB):
                        av = new_av()
                        trail = None
                        if pr == NPAIR - 1 and qb > 0:
                            trail = [(sc, dmh)
                                     for sc in range((qb - 1) * 4, qb * 4)
                                     for dmh in range(2)]
                        attn_chunk(pr, qb, 0, NSC, av, trail=trail)
                        attn_norm(pr, qb, av)
                    if pr == NPAIR - 1:
                        for sc in range((NQB - 1) * 4, NQB * 4):
                            for dmh in range(2):
                                out_group(sc, dmh)

    nc.compile()
    return nc


_NC_CACHE = None


def _get_nc():
    global _NC_CACHE
    if _NC_CACHE is None:
        _NC_CACHE = build_kernel()
    return _NC_CACHE


def kernel(x, Wq, Wk, Wv, Wo):
    x = np.asarray(x, dtype=np.float32)
    Wq = np.asarray(Wq, dtype=np.float32)
    Wk = np.asarray(Wk, dtype=np.float32)
    Wv = np.asarray(Wv, dtype=np.float32)
    Wo = np.asarray(Wo, dtype=np.float32)
    B = x.shape[0]
    ones = np.ones((P, P), dtype=np.float32)

    in_maps = []
    for core in range(8):
        b, g = divmod(core, 2)
        hs = g * HPC
        xt = np.ascontiguousarray(x[b].T)
        wq = np.stack([
            np.concatenate([Wq[hs + 2 * p], Wq[hs + 2 * p + 1]], axis=1)
            for p in range(NPAIR)])
        wk = np.stack([
            np.concatenate([Wk[hs + 2 * p], Wk[hs + 2 * p + 1]], axis=1)
            for p in range(NPAIR)])
        wv = np.concatenate([Wv[hs + h] for h in range(HPC)], axis=1)
        wo = np.ascontiguousarray(Wo[hs * DK:(hs + HPC) * DK, :])
        in_maps.append({"xt": xt, "wq": wq, "wk": wk, "wv": wv, "wo": wo,
                        "ones": ones})

    nc = _get_nc()
    res = run_bass_kernel_spmd(nc, in_maps, core_ids=list(range(8))).results

    out = np.empty((B, S, D), dtype=np.float32)
    for b in range(B):
        out[b] = res[2 * b]["out"] + res[2 * b + 1]["out"]
    return out
